# revision 6
# baseline (speedup 1.0000x reference)
"""Trainium2 Bass kernel for the mesh-Laplacian consistency loss.

Reference computation (B=16 batches, N=100000 points, M=9 neighbor slots):
    lap(pc)[b,i,:] = deg[i] * pc[b,i,:] - sum_{j=1..8} pc_ext[b, nb[i,j], :]
    out = mean(|lap(gt) - lap(pred)|)

The Laplacian is linear in pc, so lap(gt) - lap(pred) = lap(gt - pred).
We compute d = gt - pred once (host-side input marshalling), lay it out as a
node-major fp16 table [N+1, 48] (48 = 16 batches x 3 coords; row N is the
zero row used by padding indices), and run the message-passing gather on
device:

  - 8 NeuronCores, each owning 12500 nodes (padded to 12544 = 128 x 98).
  - Per core: 8 indirect DMA gathers (one per neighbor slot) of 12544 rows x
    96B from its replicated DRAM copy of the table into SBUF.
  - DVE tree-adds the 8 slot buffers -> neighbor sum NS.
  - DVE computes L = selfd * deg - NS (deg broadcast along channels).
  - ACT computes |L| and the per-partition running sum (accum_out).
  - Each core outputs 128 partial sums; host adds them and divides.
"""

import numpy as np

B = 16
N = 100000
NCORES = 8
P = 128
T = 98                    # nodes per partition per core
LOCAL = P * T             # 12544 padded nodes per core
REAL = N // NCORES        # 12500 real nodes per core
CH = B * 3                # 48 channels per node row
ZROW = N                  # zero row index (padding neighbors)
PAD_N = 100352            # padded table rows >= 7*REAL + LOCAL = 100044

_CACHE = {}


def _build_module(null=False):
    """Build the SPMD module. null=True builds a timing-baseline variant with
    identical I/O but only one gather instruction (for wall differentials)."""
    import concourse.bacc as bacc
    import concourse.bass as bass
    import concourse.tile as tile
    from concourse import mybir

    fp16 = mybir.dt.float16
    f32 = mybir.dt.float32
    i32 = mybir.dt.int32

    nc = bacc.Bacc("TRN2", target_bir_lowering=False, debug=False,
                   num_devices=NCORES)

    table_d = nc.dram_tensor("table", [PAD_N, CH], fp16, kind="ExternalInput")
    selfd_d = nc.dram_tensor("selfd", [P, T * CH], fp16, kind="ExternalInput")
    deg_d = nc.dram_tensor("deg", [P, T], fp16, kind="ExternalInput")
    idx_d = nc.dram_tensor("idx", [P, 8 * T], i32, kind="ExternalInput")
    out_d = nc.dram_tensor("acc", [P, 1], f32, kind="ExternalOutput")

    with tile.TileContext(nc) as tc:
        with tc.tile_pool(name="main", bufs=1) as pool:
            idx_t = pool.tile([P, 8 * T], i32, tag="idx")
            nc.sync.dma_start(out=idx_t[:], in_=idx_d.ap())
            deg_t = pool.tile([P, T], fp16, tag="deg")
            nc.sync.dma_start(out=deg_t[:], in_=deg_d.ap())
            selfd_t = pool.tile([P, T * CH], fp16, tag="selfd")
            nc.sync.dma_start(out=selfd_t[:], in_=selfd_d.ap())

            # One indirect DMA per (slot, node-column): the runtime's indirect
            # DMA consumes exactly one offset per partition, so each
            # instruction gathers 128 rows (one per partition) into one
            # 48-wide column slice of the slot buffer.
            nslots = 1 if null else 8
            g = []
            for j in range(nslots):
                gj = pool.tile([P, T * CH], fp16, tag=f"g{j}", name=f"g{j}")
                for t in range(T if not null else 1):
                    nc.gpsimd.indirect_dma_start(
                        out=gj[:, t * CH:(t + 1) * CH],
                        out_offset=None,
                        in_=table_d.ap(),
                        in_offset=bass.IndirectOffsetOnAxis(
                            ap=idx_t[:, j * T + t:j * T + t + 1], axis=0),
                    )
                g.append(gj)

            if not null:
                # tree-sum the 8 slot buffers on DVE
                add = nc.vector.tensor_add
                add(out=g[0][:], in0=g[0][:], in1=g[1][:])
                add(out=g[2][:], in0=g[2][:], in1=g[3][:])
                add(out=g[4][:], in0=g[4][:], in1=g[5][:])
                add(out=g[6][:], in0=g[6][:], in1=g[7][:])
                add(out=g[0][:], in0=g[0][:], in1=g[2][:])
                add(out=g[4][:], in0=g[4][:], in1=g[6][:])
                add(out=g[0][:], in0=g[0][:], in1=g[4][:])

            # center term: C = selfd * deg (deg broadcast along channels)
            deg_b = deg_t[:].to_broadcast([P, T, CH])
            nc.vector.tensor_tensor(
                out=selfd_t[:].rearrange("p (t c) -> p t c", c=CH),
                in0=selfd_t[:].rearrange("p (t c) -> p t c", c=CH),
                in1=deg_b,
                op=mybir.AluOpType.mult,
            )
            # L = C - NS
            nc.vector.tensor_tensor(
                out=g[0][:], in0=selfd_t[:], in1=g[0][:],
                op=mybir.AluOpType.subtract,
            )

            # |L| with per-partition accumulation on ACT
            abs_t = pool.tile([P, T * CH], fp16, tag="abs")
            acc_t = pool.tile([P, 1], f32, tag="acc")
            nc.scalar.activation(
                out=abs_t[:], in_=g[0][:],
                func=mybir.ActivationFunctionType.Abs,
                accum_out=acc_t[:],
            )
            nc.sync.dma_start(out=out_d.ap(), in_=acc_t[:])

    nc.compile()
    return nc


def _get_module():
    if "nc" not in _CACHE:
        _CACHE["nc"] = _build_module()
    return _CACHE["nc"]


def _prep_inputs(gt_pc, predict_pc, neighbor_id_lstlst, neighbor_num_lst):
    """Host-side sharding/marshalling: build per-core input maps."""
    gt = np.asarray(gt_pc, dtype=np.float32)
    pr = np.asarray(predict_pc, dtype=np.float32)
    nb = np.asarray(neighbor_id_lstlst).astype(np.int32, copy=False)
    deg = np.asarray(neighbor_num_lst, dtype=np.float32)

    d = gt - pr                                    # [B, N, 3] f32
    d16 = d.astype(np.float16)
    table = np.zeros((PAD_N, CH), dtype=np.float16)
    table[:N] = np.ascontiguousarray(d16.transpose(1, 0, 2)).reshape(N, CH)
    # rows N.. stay zero (padding target)

    nbp = np.full((PAD_N, 9), ZROW, dtype=np.int32)
    nbp[:N] = nb
    degp = np.zeros(PAD_N, dtype=np.float16)
    degp[:N] = deg.astype(np.float16)

    # local node n = p*T + t; nodes n >= REAL are padding and must contribute 0
    pad_mask = (np.arange(LOCAL) >= REAL).reshape(P, T)

    in_maps = []
    for c in range(NCORES):
        lo = c * REAL
        blk_idx = nbp[lo:lo + LOCAL].reshape(P, T, 9).copy()
        blk_idx[pad_mask] = ZROW          # pad nodes gather only zero rows
        idx_c = np.ascontiguousarray(
            blk_idx[:, :, 1:9].transpose(0, 2, 1)).reshape(P, 8 * T)
        deg_c = degp[lo:lo + LOCAL].reshape(P, T).copy()
        deg_c[pad_mask] = 0               # pad nodes get no center term
        selfd_c = np.ascontiguousarray(
            table[lo:lo + LOCAL].reshape(P, T * CH))
        in_maps.append({
            "table": table,
            "selfd": selfd_c,
            "deg": deg_c,
            "idx": idx_c,
        })
    return in_maps


def kernel(gt_pc, predict_pc, neighbor_id_lstlst, neighbor_num_lst):
    from concourse.bass_utils import run_bass_kernel_spmd

    nc = _get_module()
    in_maps = _prep_inputs(gt_pc, predict_pc, neighbor_id_lstlst,
                           neighbor_num_lst)
    res = run_bass_kernel_spmd(nc, in_maps, core_ids=list(range(NCORES)))
    total = 0.0
    for c in range(NCORES):
        total += float(res.results[c]["acc"].astype(np.float64).sum())
    return np.float32(total / (B * N * 3))
